# revision 1
# baseline (speedup 1.0000x reference)
"""Llama decode block (single token) on 8 TRN2 NeuronCores, tensor-parallel.

Sharding (per sharding_hint): w_q/w_k/w_v/w_ff1 column-sharded, w_o/w_ff2
row-sharded, KV cache sharded by head (4 heads/core). AllReduce after the
attention output projection and after w_ff2. The residual stream x is folded
into the all-reduces as x/8 per core, so each AR output is the full residual
sum directly.

Per-core dataflow (all matvecs run on the PE with the activation vector as
the stationary operand, streaming the weights as the moving operand):
  rmsnorm(x) -> h_cols[128,32]  (PE transpose of [32,128] rows)
  q/k/v[1,512] = h @ W         (32 k-blocks accumulated in PSUM)
  RoPE on q,k rows; q pre-scaled by 1/sqrt(128)
  scores: per 128-token tile, fused DVE multiply+reduce against K tiles
  softmax without max-subtraction (scores are O(8), exp is safe in f32)
  attn@V: per-tile PE matmuls, V tile stationary -> o[128(d),4(h)] cols
  o @ w_o + x/8 -> [1,4096] -> AllReduce #1 -> x2
  rmsnorm(x2) -> h2_cols; silu(h2 @ w_ff1) -> a[128,11] cols
  a @ w_ff2 + x2/8 -> [1,4096] -> AllReduce #2 -> output
"""

import math

import numpy as np

import concourse.bass as bass
import concourse.mybir as mybir
import concourse.tile as tile
from concourse import bacc
from concourse import bass_utils

F32 = mybir.dt.float32
AF = mybir.ActivationFunctionType
ALU = mybir.AluOpType

HIDDEN = 4096
N_HEADS = 32
HEAD_DIM = 128
INTERM = 11008
KV_LEN = 4096
N_CORES = 8

HEADS_PC = N_HEADS // N_CORES          # 4 heads per core
QKV_N = HEADS_PC * HEAD_DIM            # 512
FF_N = INTERM // N_CORES               # 1376
KB = HIDDEN // 128                     # 32 k-blocks of the hidden dim
T_TILES = KV_LEN // 128                # 32 token tiles
FF_KB_SIZES = [128] * 10 + [96]        # 1376 = 10*128 + 96
SCALE = 1.0 / math.sqrt(HEAD_DIM)


def _emit(nc, tc):
    i = {}  # dram input handles

    def din(name, shape):
        i[name] = nc.dram_tensor(name, list(shape), F32, kind="ExternalInput").ap()

    din("x", [HIDDEN])
    din("attn_norm", [HIDDEN])
    din("ffn_norm", [HIDDEN])
    din("sin", [HEAD_DIM // 2])
    din("ident32", [32, 32])
    din("cos", [HEAD_DIM // 2])
    din("wq", [HIDDEN, QKV_N])
    din("wk", [HIDDEN, QKV_N])
    din("wv", [HIDDEN, QKV_N])
    din("wo", [QKV_N, HIDDEN])
    din("kc", [KV_LEN, QKV_N])
    din("vc", [KV_LEN, QKV_N])
    din("wf1", [HIDDEN, FF_N])
    din("wf2", [FF_N, HIDDEN])
    y = nc.dram_tensor("y", [HIDDEN], F32, kind="ExternalOutput").ap()

    with (
        tc.tile_pool(name="const", bufs=1) as cpool,
        tc.tile_pool(name="wbig", bufs=4) as wpool,
        tc.tile_pool(name="kpool", bufs=2) as kpool,
        tc.tile_pool(name="vpool", bufs=2) as vpool,
        tc.tile_pool(name="sm", bufs=1) as sm,
        tc.tile_pool(name="scr", bufs=4) as scr,
        tc.tile_pool(name="psum", bufs=8, space="PSUM") as pp,
        tc.tile_pool(name="dram", bufs=1, space="DRAM") as dram,
    ):
        # ---- constants ----
        ones32 = cpool.tile([32, 1], F32)
        nc.vector.memset(ones32[:], 1.0)
        ones128 = cpool.tile([128, 1], F32)
        nc.vector.memset(ones128[:], 1.0)
        eighth = cpool.tile([1, 1], F32)
        nc.vector.memset(eighth[:], 1.0 / N_CORES)
        eps11 = cpool.tile([1, 1], F32)
        nc.vector.memset(eps11[:], 1e-6)
        ident32 = cpool.tile([32, 32], F32)
        nc.sync.dma_start(ident32[:], i["ident32"])
        ones_r32 = cpool.tile([1, 32], F32)
        nc.vector.memset(ones_r32[:], 1.0)
        ones_r128 = cpool.tile([1, 128], F32)
        nc.vector.memset(ones_r128[:], 1.0)

        sin_sb = cpool.tile([1, 64], F32)
        cos_sb = cpool.tile([1, 64], F32)
        nc.sync.dma_start(sin_sb[:], i["sin"].rearrange("(a d) -> a d", a=1))
        nc.sync.dma_start(cos_sb[:], i["cos"].rearrange("(a d) -> a d", a=1))
        sinq = cpool.tile([1, 64], F32)
        cosq = cpool.tile([1, 64], F32)
        nc.vector.tensor_scalar_mul(sinq[:], sin_sb[:], SCALE)
        nc.vector.tensor_scalar_mul(cosq[:], cos_sb[:], SCALE)

        # ---- rmsnorm #1 -> h_cols [128, 32] ----
        def rmsnorm_cols(x_dram, norm_dram, tag):
            x_rows = sm.tile([32, 128], F32, name=f"x_rows_{tag}", tag="x_rows")
            nrm_rows = sm.tile([32, 128], F32, name=f"nrm_rows_{tag}", tag="nrm_rows")
            nc.sync.dma_start(x_rows[:], x_dram.rearrange("(a d) -> a d", a=32))
            nc.sync.dma_start(nrm_rows[:], norm_dram.rearrange("(a d) -> a d", a=32))
            sq = sm.tile([32, 128], F32, name=f"sq_{tag}", tag="sq")
            ssq = sm.tile([32, 1], F32, name=f"ssq_{tag}", tag="ssq")
            nc.scalar.activation(sq[:], x_rows[:], AF.Square, accum_out=ssq[:])
            ms_psum = pp.tile([1, 1], F32, name=f"ms_psum_{tag}", tag="ps")
            nc.tensor.matmul(ms_psum[:], ones32[:], ssq[:])
            rstd = sm.tile([1, 1], F32, name=f"rstd_{tag}", tag="rstd")
            # sqrt(mean + eps), then reciprocal (Rsqrt activation is banned)
            nc.scalar.activation(rstd[:], ms_psum[:], AF.Sqrt,
                                 bias=eps11[:], scale=1.0 / HIDDEN)
            nc.vector.reciprocal(rstd[:], rstd[:])
            rstd_ps = pp.tile([32, 1], F32, name=f"rstd_ps_{tag}", tag="ps")
            nc.tensor.matmul(rstd_ps[:], ones_r32[:], rstd[:])
            rstd32 = sm.tile([32, 1], F32, name=f"rstd32_{tag}", tag="rstd32")
            nc.vector.tensor_copy(rstd32[:], rstd_ps[:])
            h_rows = sm.tile([32, 128], F32, name=f"h_rows_{tag}", tag="h_rows")
            nc.vector.tensor_tensor(h_rows[:], x_rows[:], nrm_rows[:], ALU.mult)
            nc.vector.tensor_scalar_mul(h_rows[:], h_rows[:], rstd32[:])
            h_psum = pp.tile([128, 32], F32, name=f"h_psum_{tag}", tag="ps")
            nc.tensor.transpose(h_psum[:], h_rows[:], ident32[:])
            h_cols = sm.tile([128, 32], F32, name=f"h_cols_{tag}", tag="hcols")
            nc.vector.tensor_copy(h_cols[:], h_psum[:])
            return h_cols

        h_cols = rmsnorm_cols(i["x"], i["attn_norm"], "a")

        # ---- q/k/v = h @ W (h stationary, weights moving) ----
        qkv_rows = {}
        for wname in ("wq", "wk", "wv"):
            ps = pp.tile([1, QKV_N], F32, name=f"ps_{wname}", tag="ps")
            for t in range(4):
                wt = wpool.tile([128, 8, 512], F32, name=f"{wname}_t", tag="w")
                nc.sync.dma_start(
                    wt[:],
                    i[wname][t * 1024:(t + 1) * 1024, :].rearrange(
                        "(b p) c -> p b c", p=128),
                )
                for b in range(8):
                    kb = t * 8 + b
                    nc.tensor.matmul(
                        ps[:], h_cols[:, kb:kb + 1], wt[:, b, :],
                        start=(kb == 0), stop=(kb == KB - 1),
                    )
            row = sm.tile([1, QKV_N], F32, name=f"{wname}_row")
            nc.scalar.copy(row[:], ps[:])
            qkv_rows[wname] = row

        # ---- RoPE on q (pre-scaled by 1/sqrt(d)) and k ----
        def rope(row, cos_t, sin_t, tag):
            out = sm.tile([1, QKV_N], F32, name=f"rope_{tag}")
            tmp = sm.tile([1, QKV_N], F32, name=f"rope_tmp_{tag}")
            r3 = row[:].rearrange("a (h d) -> a h d", h=HEADS_PC)
            o3 = out[:].rearrange("a (h d) -> a h d", h=HEADS_PC)
            t3 = tmp[:].rearrange("a (h d) -> a h d", h=HEADS_PC)
            x1, x2 = r3[:, :, 0:64], r3[:, :, 64:128]
            cb = cos_t[:].unsqueeze(1).to_broadcast((1, HEADS_PC, 64))
            sb = sin_t[:].unsqueeze(1).to_broadcast((1, HEADS_PC, 64))
            nc.vector.tensor_tensor(o3[:, :, 0:64], x1, cb, ALU.mult)
            nc.vector.tensor_tensor(t3[:, :, 0:64], x2, sb, ALU.mult)
            nc.vector.tensor_sub(o3[:, :, 0:64], o3[:, :, 0:64], t3[:, :, 0:64])
            nc.vector.tensor_tensor(o3[:, :, 64:128], x2, cb, ALU.mult)
            nc.vector.tensor_tensor(t3[:, :, 64:128], x1, sb, ALU.mult)
            nc.vector.tensor_add(o3[:, :, 64:128], o3[:, :, 64:128],
                                 t3[:, :, 64:128])
            return out

        q_rot = rope(qkv_rows["wq"], cosq, sinq, "q")
        k_rot = rope(qkv_rows["wk"], cos_sb, sin_sb, "k")
        v_row = qkv_rows["wv"]

        q_rep = sm.tile([128, QKV_N], F32, name="q_rep")
        qrep_ps = pp.tile([128, QKV_N], F32, name="qrep_ps", tag="ps")
        nc.tensor.matmul(qrep_ps[:], ones_r128[:], q_rot[:])
        nc.vector.tensor_copy(q_rep[:], qrep_ps[:])

        # ---- attention over the KV cache ----
        o_psum = pp.tile([128, HEADS_PC], F32, name="o_psum", tag="ps")
        denom_acc = sm.tile([128, HEADS_PC], F32, name="denom_acc")
        nc.vector.memset(denom_acc[:], 0.0)

        for st in range(4):
            k_sup = kpool.tile([128, 8, 512], F32, name="k_sup", tag="k")
            v_sup = vpool.tile([128, 8, 512], F32, name="v_sup", tag="v")
            nc.sync.dma_start(
                k_sup[:],
                i["kc"][st * 1024:(st + 1) * 1024, :].rearrange(
                    "(b p) c -> p b c", p=128),
            )
            nc.sync.dma_start(
                v_sup[:],
                i["vc"][st * 1024:(st + 1) * 1024, :].rearrange(
                    "(b p) c -> p b c", p=128),
            )
            for b in range(8):
                gt = st * 8 + b
                scores = scr.tile([128, HEADS_PC], F32, name="scores", tag="sc")
                scratch = scr.tile([128, QKV_N], F32, name="scratch", tag="scratch")
                nc.vector.tensor_tensor(scratch[:], k_sup[:, b, :], q_rep[:],
                                        ALU.mult)
                nc.vector.tensor_reduce(
                    scores[:],
                    scratch[:].rearrange("p (h d) -> p h d", h=HEADS_PC),
                    mybir.AxisListType.X, ALU.add)
                expt = scr.tile([128, HEADS_PC], F32, name="expt", tag="expt")
                nc.scalar.activation(expt[:], scores[:], AF.Exp)
                nc.vector.tensor_add(denom_acc[:], denom_acc[:], expt[:])
                for h in range(HEADS_PC):
                    # start clears has_written for the whole PSUM bank, so
                    # only the very first matmul into o_psum may set it.
                    nc.tensor.matmul(
                        o_psum[:, h:h + 1],
                        v_sup[:, b, h * 128:(h + 1) * 128],
                        expt[:, h:h + 1],
                        start=(gt == 0 and h == 0), stop=False,
                    )

        # current-token contribution (position KV_LEN)
        s_new = sm.tile([1, HEADS_PC], F32, name="s_new")
        scr_new = sm.tile([1, QKV_N], F32, name="scr_new")
        nc.vector.tensor_tensor(scr_new[:], q_rot[:], k_rot[:], ALU.mult)
        nc.vector.tensor_reduce(
            s_new[:],
            scr_new[:].rearrange("a (h d) -> a h d", h=HEADS_PC),
            mybir.AxisListType.X, ALU.add)
        e_new = sm.tile([1, HEADS_PC], F32, name="e_new")
        nc.scalar.activation(e_new[:], s_new[:], AF.Exp)
        for h in range(HEADS_PC):
            nc.tensor.matmul(
                o_psum[:, h:h + 1],
                v_row[:, h * 128:(h + 1) * 128],
                e_new[:, h:h + 1],
                start=False, stop=(h == HEADS_PC - 1),
            )

        denom_psum = pp.tile([1, HEADS_PC], F32, name="denom_psum", tag="ps")
        nc.tensor.matmul(denom_psum[:], ones128[:], denom_acc[:])
        denom = sm.tile([1, HEADS_PC], F32, name="denom")
        nc.vector.tensor_copy(denom[:], denom_psum[:])
        nc.vector.tensor_add(denom[:], denom[:], e_new[:])
        nc.vector.reciprocal(denom[:], denom[:])
        recip_ps = pp.tile([128, HEADS_PC], F32, name="recip_ps", tag="ps")
        nc.tensor.matmul(recip_ps[:], ones_r128[:], denom[:])
        recip_bc = sm.tile([128, HEADS_PC], F32, name="recip_bc")
        nc.vector.tensor_copy(recip_bc[:], recip_ps[:])
        o_sb = sm.tile([128, HEADS_PC], F32, name="o_sb")
        nc.vector.tensor_tensor(o_sb[:], o_psum[:], recip_bc[:], ALU.mult)

        # ---- o @ w_o + x/8 -> [1,4096] -> AllReduce #1 ----
        x_row = sm.tile([1, HIDDEN], F32, name="x_row", tag="xrow")
        nc.sync.dma_start(x_row[:], i["x"].rearrange("(a d) -> a d", a=1))

        chunks1 = [pp.tile([1, 512], F32, name=f"c1_{n}", tag="ps")
                   for n in range(8)]
        for kb in range(HEADS_PC):
            wo_t = wpool.tile([128, HIDDEN], F32, name="wo_t", tag="w")
            nc.sync.dma_start(wo_t[:], i["wo"][kb * 128:(kb + 1) * 128, :])
            for n in range(8):
                nc.tensor.matmul(
                    chunks1[n][:], o_sb[:, kb:kb + 1],
                    wo_t[:, n * 512:(n + 1) * 512],
                    start=(kb == 0), stop=False,
                )
        o_row = sm.tile([1, HIDDEN], F32, name="o_row", tag="outrow")
        for n in range(8):
            nc.tensor.matmul(
                chunks1[n][:], eighth[:], x_row[:, n * 512:(n + 1) * 512],
                start=False, stop=True,
            )
            nc.scalar.copy(o_row[:, n * 512:(n + 1) * 512], chunks1[n][:])

        ar1_in = dram.tile([HIDDEN], F32, name="ar1_in")
        ar1_out = dram.tile([HIDDEN], F32, name="ar1_out")
        nc.sync.dma_start(ar1_in[:], o_row[:])
        nc.gpsimd.collective_compute(
            "AllReduce", ALU.add,
            replica_groups=[list(range(N_CORES))],
            ins=[ar1_in[:].opt()], outs=[ar1_out[:].opt()],
        )

        # ---- MLP ----
        h2_cols = rmsnorm_cols(ar1_out[:], i["ffn_norm"], "b")
        x2_row = sm.tile([1, HIDDEN], F32, name="x2_row", tag="xrow")
        nc.sync.dma_start(x2_row[:], ar1_out[:].rearrange("(a d) -> a d", a=1))

        # two tiles (separate banks): start/stop must cover a consistent
        # partition count per zero region, and the 96-row tail block differs.
        f1a = pp.tile([128, 10], F32, name="f1a", tag="ps")
        f1b = pp.tile([96, 1], F32, name="f1b", tag="ps")
        for t in range(16):
            wt = wpool.tile([128, 2, FF_N], F32, name="wf1_t", tag="w")
            nc.sync.dma_start(
                wt[:],
                i["wf1"][t * 256:(t + 1) * 256, :].rearrange(
                    "(b p) c -> p b c", p=128),
            )
            for half in range(2):
                kb = 2 * t + half
                for mb in range(11):
                    sz = FF_KB_SIZES[mb]
                    out = f1a[:, mb:mb + 1] if mb < 10 else f1b[:]
                    nc.tensor.matmul(
                        out,
                        wt[:, half, mb * 128:mb * 128 + sz],
                        h2_cols[:, kb:kb + 1],
                        start=(kb == 0 and mb in (0, 10)),
                        stop=(kb == KB - 1 and mb in (9, 10)),
                    )
        a_sb = sm.tile([128, 11], F32, name="a_sb")
        sig = sm.tile([128, 11], F32, name="sig")
        # silu(x) = x * sigmoid(x)
        nc.scalar.activation(sig[0:96, 10:11], f1b[:], AF.Sigmoid)
        nc.scalar.activation(sig[:, 0:10], f1a[:], AF.Sigmoid)
        nc.vector.tensor_tensor(a_sb[0:96, 10:11], f1b[:],
                                sig[0:96, 10:11], ALU.mult)
        nc.vector.tensor_tensor(a_sb[:, 0:10], f1a[:],
                                sig[:, 0:10], ALU.mult)

        chunks2 = [pp.tile([1, 512], F32, name=f"c2_{n}", tag="ps")
                   for n in range(8)]
        for kb in range(11):
            sz = FF_KB_SIZES[kb]
            wt = wpool.tile([sz, HIDDEN], F32, name="wf2_t", tag="w")
            nc.sync.dma_start(wt[:], i["wf2"][kb * 128:kb * 128 + sz, :])
            for n in range(8):
                nc.tensor.matmul(
                    chunks2[n][:], a_sb[0:sz, kb:kb + 1],
                    wt[:, n * 512:(n + 1) * 512],
                    start=(kb == 0), stop=False,
                )
        ff_row = sm.tile([1, HIDDEN], F32, name="ff_row", tag="outrow")
        for n in range(8):
            nc.tensor.matmul(
                chunks2[n][:], eighth[:], x2_row[:, n * 512:(n + 1) * 512],
                start=False, stop=True,
            )
            nc.scalar.copy(ff_row[:, n * 512:(n + 1) * 512], chunks2[n][:])

        ar2_in = dram.tile([HIDDEN], F32, name="ar2_in")
        ar2_out = dram.tile([HIDDEN], F32, name="ar2_out")
        nc.sync.dma_start(ar2_in[:], ff_row[:])
        nc.gpsimd.collective_compute(
            "AllReduce", ALU.add,
            replica_groups=[list(range(N_CORES))],
            ins=[ar2_in[:].opt()], outs=[ar2_out[:].opt()],
        )
        nc.sync.dma_start(y[:], ar2_out[:])


_BUILT = None


def _build():
    global _BUILT
    if _BUILT is None:
        nc = bacc.Bacc("TRN2", target_bir_lowering=False, debug=False,
                       num_devices=N_CORES)
        with tile.TileContext(nc) as tc:
            _emit(nc, tc)
        nc.compile()
        _BUILT = nc
    return _BUILT


def _shard(inputs):
    f = lambda a: np.ascontiguousarray(np.asarray(a, dtype=np.float32))
    x = f(inputs["x"])
    attn_norm = f(inputs["attn_norm"])
    ffn_norm = f(inputs["ffn_norm"])
    pos = int(np.asarray(inputs["pos"]))
    sin = f(inputs["sin_cache"][pos])
    cos = f(inputs["cos_cache"][pos])
    wq, wk, wv = f(inputs["w_q"]), f(inputs["w_k"]), f(inputs["w_v"])
    wo, wf1, wf2 = f(inputs["w_o"]), f(inputs["w_ff1"]), f(inputs["w_ff2"])
    kc = f(inputs["k_cache"]).reshape(KV_LEN, N_HEADS * HEAD_DIM)
    vc = f(inputs["v_cache"]).reshape(KV_LEN, N_HEADS * HEAD_DIM)

    in_maps = []
    for c in range(N_CORES):
        qs = slice(c * QKV_N, (c + 1) * QKV_N)
        fs = slice(c * FF_N, (c + 1) * FF_N)
        in_maps.append({
            "x": x,
            "ident32": np.eye(32, dtype=np.float32),
            "attn_norm": attn_norm,
            "ffn_norm": ffn_norm,
            "sin": sin,
            "cos": cos,
            "wq": np.ascontiguousarray(wq[:, qs]),
            "wk": np.ascontiguousarray(wk[:, qs]),
            "wv": np.ascontiguousarray(wv[:, qs]),
            "wo": np.ascontiguousarray(wo[qs, :]),
            "kc": np.ascontiguousarray(kc[:, qs]),
            "vc": np.ascontiguousarray(vc[:, qs]),
            "wf1": np.ascontiguousarray(wf1[:, fs]),
            "wf2": np.ascontiguousarray(wf2[fs, :]),
        })
    return in_maps


def kernel(**inputs):
    nc = _build()
    in_maps = _shard(inputs)
    res = bass_utils.run_bass_kernel_spmd(
        nc, in_maps, core_ids=list(range(N_CORES)))
    return res.results[0]["y"]

